# revision 12
# baseline (speedup 1.0000x reference)
"""Sliding-window GQA causal self-attention block for 8 trn2 NeuronCores.

Sharding: batch (4) x T-halves (2) -> 8 cores, no collectives. Each core gets
x.T for its T-half plus a 256-row key/value halo and computes its (1024, 1024)
slice of the output.

v2 design notes (cost-model driven):
- qkv projection runs as fp8e4 hi+lo DoubleRow matmuls (x and w split into
  fp8 hi/lo pairs on the host; 3 DR terms replace 4 bf16 matmuls per
  256-channel pair -> 25% fewer PE rows at bf16-level accuracy).
- band masks are added into the scores PSUM by bf16 identity-stationary
  matmuls; replaces the DVE mask multiplies and the gpsimd partition
  broadcasts entirely.
- exp runs as ONE activation per (qb, g) over [128, 4*3*128] f32 PSUM.
- att@v is "flipped": stationary = exp'd scores, moving = v (65 cols incl a
  ones column), so cost is 65 rows per key block instead of 128, and the
  softmax denominator lands as a per-query PSUM column -> cheap [128, 4]
  reciprocal + stride-0-broadcast normalize on DVE.
- normalized y ([q, d] layout) is transposed back to [d, q] via PE transpose
  for the row-major output projection (bf16).
"""

import dataclasses

import numpy as np
import ml_dtypes

import concourse.bass as bass
import concourse.mybir as mybir
import concourse.tile as tile
from concourse import bacc
from concourse.bass_utils import run_bass_kernel_spmd

F8 = ml_dtypes.float8_e4m3fn
BF = ml_dtypes.bfloat16
F32 = mybir.dt.float32
BF16 = mybir.dt.bfloat16
FP8 = mybir.dt.float8e4
DR = mybir.MatmulPerfMode.DoubleRow

B, T, C = 4, 2048, 1024
H, KV, HD = 16, 4, 64
WIN = 256
TL = T // 2            # 1024 own rows per core
TH = TL + WIN          # 1280 with halo
NEG = -28672.0         # additive mask value (bf16)
WS = 32.0              # host-side weight prescale for fp8 dynamic range


def _build_program():
    nc = bacc.Bacc("TRN2", target_bir_lowering=False, debug=False, num_devices=8)
    dt = mybir.dt
    xh = nc.dram_tensor("xh", [8, 128, TH], dt.float8e4, kind="ExternalInput").ap()
    xl = nc.dram_tensor("xl", [8, 128, TH], dt.float8e4, kind="ExternalInput").ap()
    wqkh = nc.dram_tensor("wqkh", [8, 128, 1280], dt.float8e4, kind="ExternalInput").ap()
    wqkl = nc.dram_tensor("wqkl", [8, 128, 1280], dt.float8e4, kind="ExternalInput").ap()
    wvh = nc.dram_tensor("wvh", [8, 128, 256], dt.float8e4, kind="ExternalInput").ap()
    wvl = nc.dram_tensor("wvl", [8, 128, 256], dt.float8e4, kind="ExternalInput").ap()
    wp = nc.dram_tensor("wp", [8, 128, C], dt.bfloat16, kind="ExternalInput").ap()
    cq = nc.dram_tensor("cq", [2, 128, TL], dt.bfloat16, kind="ExternalInput").ap()
    ck = nc.dram_tensor("ck", [2, 128, TH], dt.bfloat16, kind="ExternalInput").ap()
    mb = nc.dram_tensor("mb", [128, 4, 128], dt.bfloat16, kind="ExternalInput").ap()
    idt = nc.dram_tensor("idt", [128, 128], dt.bfloat16, kind="ExternalInput").ap()
    out = nc.dram_tensor("out", [TL, C], dt.bfloat16, kind="ExternalOutput").ap()

    with tile.TileContext(nc) as tc:
        _kernel_body(tc, nc, xh, xl, wqkh, wqkl, wvh, wvl, wp, cq, ck,
                     mb, idt, out)
    nc.compile()
    return nc


def _kernel_body(tc, nc, xh, xl, wqkh, wqkl, wvh, wvl, wp, cq, ck,
                 mb, idt, out, dbg=None):
    import contextlib
    ctx = contextlib.ExitStack()
    with ctx:
        consts = ctx.enter_context(tc.tile_pool(name="consts", bufs=1))
        persist = ctx.enter_context(tc.tile_pool(name="persist", bufs=1))

        # ---- load persistent inputs ----
        xh_sb = persist.tile([128, 8, TH], FP8, tag="xh")
        xl_sb = persist.tile([128, 8, TH], FP8, tag="xl")
        wqh_sb = persist.tile([128, 8, 1280], FP8, tag="wqh")
        wql_sb = persist.tile([128, 8, 1280], FP8, tag="wql")
        wvh_sb = persist.tile([128, 8, 256], FP8, tag="wvh")
        wvl_sb = persist.tile([128, 8, 256], FP8, tag="wvl")
        wp_sb = persist.tile([128, 8, C], BF16, tag="wp")
        for kc in range(8):
            nc.sync.dma_start(out=xh_sb[:, kc, :], in_=xh[kc])
            nc.sync.dma_start(out=xl_sb[:, kc, :], in_=xl[kc])
            nc.sync.dma_start(out=wqh_sb[:, kc, :], in_=wqkh[kc])
            nc.sync.dma_start(out=wql_sb[:, kc, :], in_=wqkl[kc])
        for kc in range(8):
            nc.sync.dma_start(out=wvh_sb[:, kc, :], in_=wvh[kc])
            nc.sync.dma_start(out=wvl_sb[:, kc, :], in_=wvl[kc])
            nc.sync.dma_start(out=wp_sb[:, kc, :], in_=wp[kc])
        cq_sb = consts.tile([128, 2, TL], BF16)
        nc.sync.dma_start(out=cq_sb[:, 0, :], in_=cq[0])
        nc.sync.dma_start(out=cq_sb[:, 1, :], in_=cq[1])
        ck_sb = consts.tile([128, 2, TH], BF16)
        nc.sync.dma_start(out=ck_sb[:, 0, :], in_=ck[0])
        nc.sync.dma_start(out=ck_sb[:, 1, :], in_=ck[1])
        mb_sb = consts.tile([128, 4, 128], BF16)
        nc.gpsimd.dma_start(out=mb_sb[:], in_=mb)
        idt_sb = consts.tile([128, 128], BF16)
        nc.gpsimd.dma_start(out=idt_sb[:], in_=idt)

        # persistent compute tensors
        qT = [persist.tile([64, TL], BF16, tag=f"qT{h}", name=f"qT{h}")
              for h in range(H)]
        kT = [persist.tile([64, TH], BF16, tag=f"kT{g}", name=f"kT{g}")
              for g in range(KV)]
        v65 = persist.tile([128, 10, KV, 65], BF16, tag="v65")
        yT = persist.tile([128, 8, TL], BF16, tag="yT")

        def dr3(out_ap, p, stat_h, stat_l, stat_cols, mov_h, mov_l, mov_cols,
                first, last):
            """Three hi/lo DoubleRow terms for chunk pair p (contraction
            channels [256p, 256p+256))."""
            sh = stat_h[:, 2 * p:2 * p + 2, stat_cols[0]:stat_cols[1]]
            sl = stat_l[:, 2 * p:2 * p + 2, stat_cols[0]:stat_cols[1]]
            mh = mov_h[:, 2 * p:2 * p + 2, mov_cols[0]:mov_cols[1]]
            ml = mov_l[:, 2 * p:2 * p + 2, mov_cols[0]:mov_cols[1]]
            nc.tensor.matmul(out_ap, sh, mh, start=first, stop=False,
                             perf_mode=DR)
            nc.tensor.matmul(out_ap, sh, ml, start=False, stop=False,
                             perf_mode=DR)
            nc.tensor.matmul(out_ap, sl, mh, start=False, stop=last,
                             perf_mode=DR)

        # ======== phase 1: qkv projection + rope ========
        with tc.tile_pool(name="pps", bufs=1, space="PSUM") as pps, \
             tc.tile_pool(name="vps", bufs=2, space="PSUM") as vps, \
             tc.tile_pool(name="ropes", bufs=2) as ropes:

            def rope_pair(pe, po, cs_sb, tlen):
                e_sb = ropes.tile([128, tlen], BF16, tag="e_sb")
                o_sb = ropes.tile([128, tlen], BF16, tag="o_sb")
                nc.scalar.mul(e_sb[:], pe[:, 0:tlen], 1.0 / WS)
                nc.scalar.mul(o_sb[:], po[:, 0:tlen], 1.0 / WS)
                ne = ropes.tile([128, tlen], BF16, tag="r0")
                no_ = ropes.tile([128, tlen], BF16, tag="r1")
                t1 = ropes.tile([128, tlen], BF16, tag="r2")
                t2 = ropes.tile([128, tlen], BF16, tag="r3")
                nc.vector.tensor_mul(t1[:], e_sb[:], cs_sb[:, 0, 0:tlen])
                nc.vector.tensor_mul(t2[:], o_sb[:], cs_sb[:, 1, 0:tlen])
                nc.vector.tensor_sub(ne[:], t1[:], t2[:])
                nc.vector.tensor_mul(t1[:], e_sb[:], cs_sb[:, 1, 0:tlen])
                nc.vector.tensor_mul(t2[:], o_sb[:], cs_sb[:, 0, 0:tlen])
                nc.vector.tensor_add(no_[:], t1[:], t2[:])
                return ne, no_

            # q: wqk cols [0:512]=all-heads-evens, [512:1024]=all-heads-odds
            for c4 in range(4):
                pe = pps.tile([128, TH], F32, tag="pe")
                po = pps.tile([128, TH], F32, tag="po")
                for q4 in range(4):     # 256-token quarters
                    tcols = (WIN + q4 * 256, WIN + q4 * 256 + 256)
                    oap_e = pe[:, q4 * 256:(q4 + 1) * 256]
                    oap_o = po[:, q4 * 256:(q4 + 1) * 256]
                    for p in range(4):
                        dr3(oap_e, p, wqh_sb, wql_sb,
                            (c4 * 128, (c4 + 1) * 128),
                            xh_sb, xl_sb, tcols, p == 0, p == 3)
                    for p in range(4):
                        dr3(oap_o, p, wqh_sb, wql_sb,
                            (512 + c4 * 128, 512 + (c4 + 1) * 128),
                            xh_sb, xl_sb, tcols, p == 0, p == 3)
                ne, no_ = rope_pair(pe, po, cq_sb, TL)
                for j in range(4):
                    h = c4 * 4 + j
                    nc.sync.dma_start(out=qT[h][0:32, :],
                                      in_=ne[j * 32:(j + 1) * 32, :])
                    nc.sync.dma_start(out=qT[h][32:64, :],
                                      in_=no_[j * 32:(j + 1) * 32, :])

            # k: wqk cols [1024:1152]=kv evens, [1152:1280]=kv odds, full TH
            pe = pps.tile([128, TH], F32, tag="pe")
            po = pps.tile([128, TH], F32, tag="po")
            for q4 in range(5):
                tcols = (q4 * 256, q4 * 256 + 256)
                oap_e = pe[:, q4 * 256:(q4 + 1) * 256]
                oap_o = po[:, q4 * 256:(q4 + 1) * 256]
                for p in range(4):
                    dr3(oap_e, p, wqh_sb, wql_sb, (1024, 1152),
                        xh_sb, xl_sb, tcols, p == 0, p == 3)
                for p in range(4):
                    dr3(oap_o, p, wqh_sb, wql_sb, (1152, 1280),
                        xh_sb, xl_sb, tcols, p == 0, p == 3)
            ne, no_ = rope_pair(pe, po, ck_sb, TH)
            for g in range(KV):
                nc.sync.dma_start(out=kT[g][0:32, :],
                                  in_=ne[g * 32:(g + 1) * 32, :])
                nc.sync.dma_start(out=kT[g][32:64, :],
                                  in_=no_[g * 32:(g + 1) * 32, :])

            # v: natural layout (t partitions, 4 heads x 64) + ones column
            for tcn in range(10):
                pv = vps.tile([128, 256], F32, tag="pv")
                tc_cols = (tcn * 128, (tcn + 1) * 128)
                for p in range(4):
                    dr3(pv[:], p, xh_sb, xl_sb, tc_cols,
                        wvh_sb, wvl_sb, (0, 256), p == 0, p == 3)
                v4 = v65[:, tcn, :, 0:64]
                nc.scalar.mul(v4, pv[:].rearrange("p (g c) -> p g c", c=64),
                              1.0 / WS)
            nc.vector.memset(v65[:, :, :, 64:65], 1.0)

        if dbg is not None:
            nc.sync.dma_start(out=dbg["d_q0"], in_=qT[0][:])
            nc.sync.dma_start(out=dbg["d_q6"], in_=qT[6][:])
            nc.sync.dma_start(out=dbg["d_k0"], in_=kT[0][:])
            nc.sync.dma_start(out=dbg["d_k1"], in_=kT[1][:])
            nc.sync.dma_start(out=dbg["d_v"],
                              in_=v65[:].rearrange("p a b c -> p (a b c)"))

        # ======== phase 2: attention + interleaved output projection ========
        with tc.tile_pool(name="stps", bufs=2, space="PSUM") as stps, \
             tc.tile_pool(name="yups", bufs=1, space="PSUM") as yups, \
             tc.tile_pool(name="ops", bufs=2, space="PSUM") as ops, \
             tc.tile_pool(name="atts", bufs=3) as atts:
            for qb in range(8):
                for g in range(KV):
                    yu = yups.tile([128, 4, 96], F32, tag="yu")
                    for jh in range(2):     # head pairs within the group
                        stq = stps.tile([128, 2, 3, 128], F32, tag="stq")
                        # accumulation groups must be CONSECUTIVE PE
                        # instructions: emit each cc's score + its mask-add
                        # back to back.
                        s0 = 0 if qb <= 1 else 1       # mask slot for cc=0
                        mslot = {0: s0, 1: (3 if qb == 0 else None), 2: 2}
                        for j2 in range(2):
                            j = 2 * jh + j2
                            h = 4 * g + j
                            for cc in range(3):
                                ms = mslot[cc]
                                nc.tensor.matmul(
                                    stq[:, j2, cc, :],
                                    kT[g][:, (qb + cc) * 128:
                                          (qb + cc + 1) * 128],
                                    qT[h][:, qb * 128:(qb + 1) * 128],
                                    start=True, stop=(ms is None))
                                if ms is not None:
                                    nc.tensor.matmul(
                                        stq[:, j2, cc, :], idt_sb[:],
                                        mb_sb[:, ms, :],
                                        start=False, stop=True)
                        # exp over the (2 heads x 3 blocks) tile
                        pt = atts.tile([128, 2, 3, 128], BF16, tag="pt")
                        nc.scalar.activation(
                            pt[:].rearrange("p a b c -> p (a b c)"),
                            stq[:].rearrange("p a b c -> p (a b c)"),
                            mybir.ActivationFunctionType.Exp, scale=0.125)
                        # att@v flipped: stat = pt block, mov = v (+ones col)
                        if dbg is not None and qb == 3 and g == 1 and jh == 1:
                            nc.sync.dma_start(
                                out=dbg["d_pt"],
                                in_=pt[:].rearrange("p a b c -> p (a b c)"))
                        for j2 in range(2):
                            j = 2 * jh + j2
                            for cc in range(3):
                                nc.tensor.matmul(
                                    yu[:, j, 0:65],
                                    pt[:, j2, cc, :],
                                    v65[:, qb + cc, g, :],
                                    start=(cc == 0), stop=(cc == 2))
                    # denominators: column 64 -> reciprocal -> normalize
                    rsb = atts.tile([128, 4], F32, tag="rsb")
                    nc.vector.reciprocal(rsb[:], yu[:, :, 64])
                    yv = atts.tile([128, 4, 64], BF16, tag="yv")
                    rbc = dataclasses.replace(
                        rsb[:], ap=[rsb.ap[0], [1, 4], [0, 64]])
                    nc.vector.tensor_mul(yv[:], yu[:, :, 0:64], rbc)
                    if dbg is not None and qb == 3 and g == 1:
                        nc.sync.dma_start(out=dbg["d_rs"], in_=rsb[:])
                        nc.sync.dma_start(
                            out=dbg["d_yv"],
                            in_=yv[:].rearrange("p a b -> p (a b)"))
                    # transpose [q, d] -> [d, q] for the output projection
                    tp = yups.tile([128, 2, 128], BF16, tag="tp")
                    for jj in range(2):
                        nc.tensor.transpose(
                            tp[:, jj, :],
                            yv[:].rearrange("p a b -> p (a b)")[
                                :, jj * 128:(jj + 1) * 128],
                            idt_sb[:])
                    ydst = dataclasses.replace(
                        yT[:, 2 * g, qb * 128:(qb + 1) * 128],
                        ap=[yT.ap[0], [TL, 2], [1, 128]])
                    nc.vector.tensor_copy(ydst, tp[:])
                # output projection for this qb (t-tile == qb)
                o_sb = atts.tile([128, C], BF16, tag="o_sb")
                for oc in range(4):
                    op = ops.tile([128, 256], F32, tag="op")
                    for pr in range(8):
                        nc.tensor.matmul(
                            op[:],
                            yT[:, pr, qb * 128:(qb + 1) * 128],
                            wp_sb[:, pr, oc * 256:(oc + 1) * 256],
                            start=(pr == 0), stop=(pr == 7))
                    nc.scalar.copy(o_sb[:, oc * 256:(oc + 1) * 256], op[:])
                nc.sync.dma_start(out=out[qb * 128:(qb + 1) * 128, :],
                                  in_=o_sb[:])
            if dbg is not None:
                nc.sync.dma_start(out=dbg["d_yT"],
                                  in_=yT[:].rearrange("p a b -> p (a b)"))


_PROGRAM_CACHE = {}


def _get_program():
    if "nc" not in _PROGRAM_CACHE:
        _PROGRAM_CACHE["nc"] = _build_program()
    return _PROGRAM_CACHE["nc"]


def _hi_lo(a):
    hi = a.astype(F8)
    lo = (a - hi.astype(np.float32)).astype(F8)
    return hi, lo


def _tri_mask(kind):
    """[128, 128] additive masks over (key-in-block, query-in-block)."""
    k = np.arange(128)[:, None]
    q = np.arange(128)[None, :]
    if kind == "upper":      # cc=0 edge: visible iff k > q
        m = np.where(k > q, 0.0, NEG)
    elif kind == "lower":    # cc=2 edge: visible iff k <= q
        m = np.where(k <= q, 0.0, NEG)
    elif kind == "full_neg":
        m = np.full((128, 128), NEG)
    else:                    # zeros
        m = np.zeros((128, 128))
    return m.astype(np.float32)


def prepare_in_maps(x, freqs_cos, freqs_sin, w_attn, b_attn, w_proj, b_proj):
    x = np.asarray(x, dtype=np.float32)
    freqs_cos = np.asarray(freqs_cos, dtype=np.float32)
    freqs_sin = np.asarray(freqs_sin, dtype=np.float32)
    w_attn = np.asarray(w_attn, dtype=np.float32)
    b_attn = np.asarray(b_attn, dtype=np.float32)
    w_proj = np.asarray(w_proj, dtype=np.float32)
    assert not np.any(b_attn), "kernel assumes zero qkv bias"

    # q/k channel permutation: evens block then odds block, head-major
    qch = np.arange(H * HD).reshape(H, 32, 2)
    q_perm = np.concatenate([qch[:, :, 0].reshape(-1), qch[:, :, 1].reshape(-1)])
    kch = H * HD + np.arange(KV * HD).reshape(KV, 32, 2)
    k_perm = np.concatenate([kch[:, :, 0].reshape(-1), kch[:, :, 1].reshape(-1)])
    wqk = np.ascontiguousarray(
        w_attn[np.concatenate([q_perm, k_perm])].T) * WS     # (1024, 1280)
    wqk_h, wqk_l = _hi_lo(wqk)
    wv_f = np.ascontiguousarray(w_attn[(H + KV) * HD:].T) * WS
    wv_h, wv_l = _hi_lo(wv_f)
    wp_h = np.ascontiguousarray(w_proj.T).astype(BF)

    cos4 = np.tile(freqs_cos.T, (4, 1)).astype(np.float32)    # (128, T)
    sin4 = np.tile(freqs_sin.T, (4, 1)).astype(np.float32)

    # mask bank: slots [A: cc0 for qb<=1, B: cc0 standard, C: cc2, D: cc1 qb0]
    def mask_bank(first_half):
        a = _tri_mask("full_neg" if first_half else "upper")
        b_ = _tri_mask("upper")
        c_ = _tri_mask("lower")
        d = _tri_mask("full_neg" if first_half else "zero")
        return np.stack([a, b_, c_, d], axis=1).astype(BF)   # (128, 4, 128)

    idt = np.eye(128, dtype=np.float32).astype(BF)

    in_maps = []
    for core in range(8):
        b, h = divmod(core, 2)
        t0 = h * TL
        xs = np.zeros((TH, C), dtype=np.float32)
        lo = max(0, t0 - WIN)
        xs[TH - (t0 + TL - lo):] = x[b, lo:t0 + TL]
        xT = np.ascontiguousarray(xs.T)            # (1024, 1280)
        xT_h, xT_l = _hi_lo(xT)
        cpad = np.zeros((128, TH), dtype=np.float32)
        spad = np.zeros((128, TH), dtype=np.float32)
        cpad[:, TH - (t0 + TL - lo):] = cos4[:, lo:t0 + TL]
        spad[:, TH - (t0 + TL - lo):] = sin4[:, lo:t0 + TL]
        in_maps.append({
            "xh": xT_h.reshape(8, 128, TH), "xl": xT_l.reshape(8, 128, TH),
            "wqkh": wqk_h.reshape(8, 128, 1280),
            "wqkl": wqk_l.reshape(8, 128, 1280),
            "wvh": wv_h.reshape(8, 128, 256), "wvl": wv_l.reshape(8, 128, 256),
            "wp": wp_h.reshape(8, 128, C),
            "cq": np.stack([cos4[:, t0:t0 + TL],
                            sin4[:, t0:t0 + TL]]).astype(BF),
            "ck": np.stack([cpad, spad]).astype(BF),
            "mb": mask_bank(h == 0), "idt": idt,
        })

    return in_maps


def kernel(**inputs):
    in_maps = prepare_in_maps(**inputs)
    nc = _get_program()
    res = run_bass_kernel_spmd(nc, in_maps, list(range(8)))
    return _gather(res, np.asarray(inputs["b_proj"], dtype=np.float32))


def _gather(res, b_proj):
    out = np.empty((B, T, C), dtype=np.float32)
    for core in range(8):
        b, h = divmod(core, 2)
        out[b, h * TL:(h + 1) * TL] = np.asarray(
            res.results[core]["out"], dtype=np.float32)
    if np.any(b_proj):
        out += b_proj
    return out
